# revision 54
# baseline (speedup 1.0000x reference)
"""Trainium2 Bass kernel: out = softmax(gelu_tanh(x @ W^T), axis=-1) + bias.

Full shapes: x [8192, 4096] f32, weight [4096, 4096] f32, bias [4096] f32.
Sharding: data-parallel over rows of x across 8 NeuronCores (1024 rows/core);
weight and bias replicated.

Matmul runs in fp8e4m3 DoubleRow (2 fp8 MACs/cell/cycle = 157 TF/s peak;
measured stream spacing 216ns per 512-col 256-k matmul = 155 TF/s). Weight
values ~U(-1/64,1/64) sit at e4m3's min-normal boundary, so they are
pre-scaled x64 into [-1,1]; the scale is undone inside the fused epilogue.
End-to-end scale-relative error vs the f32 reference is ~1.2e-2 (gate 2e-2):
fp8 quantization of both operands gives ~5% rms per-term error which largely
cancels through the row softmax.

Structure per core (MC=1024 rows = 8 m-tiles of 128):
  - W [4096,4096] fp8 is kept FULLY SBUF-resident (131KB/partition),
    DMA'd once in 512-col slabs.
  - Phase 1 (m-tiles 0..2): j-outer over n-slabs, consuming each W slab as
    it lands (needs ~200GB/s of the 358GB/s DMA peak).
  - Phase 2 (m-tiles 3..7): i-outer — all 4096 columns of one m-tile are
    computed back-to-back, so its softmax row-sum completes immediately and
    the normalize + output DMA overlap the next m-tile's matmuls. Only the
    last m-tile's normalize (~6us on DVE) remains in the tail.
  - Epilogue per 128x512 psum tile: exp(gelu(v)) via ACT Square/Tanh/Exp
    (all share the exp_and_others table -> single ACT_TABLE_LOAD) and two
    DVE scalar_tensor_tensor ops; row sums accumulate via ACT accum_out.
    gelu = 0.5*v*(1+tanh(0.7978845608*(v+0.044715*v^3))) exactly as the
    reference.
  - x loads and output stores issue from the idle SP queue so they overlap
    the W issues on GpSimd; phase-1 normalize backlog drains on DVE during
    early phase-2 windows.

Measured on trn2 (8 cores): ~255-259us HW exec (best 254.9), vs 221.2us
fp8-DoubleRow matmul roofline per core; matmul stream measures 224us with
~6us DMA-ramp stalls at the head, plus ~6.5us fixed engine preamble, ~4us
ACT drain + ~7us normalize at the tail, and ~10us of framework semaphore
teardown. Scale-relative error 1.23e-2 (bf16 bias adds ~6e-4 over the
1.17e-2 of the fp8 matmul). Previous bf16 baseline: 490us; first fp8
version: 302us.

MEASUREMENT TRAP - thermal throttle: back-to-back runs can put the chip
in the P0 power state where EVERY engine runs ~1.2x slower (mm spacing
259ns instead of 216, stt 826 instead of 688) and the kernel measures
~303-305us instead of ~256. ALWAYS validate a measurement by checking
the modal matmul start-to-start spacing in the NTFF (216 = clean 2.4GHz,
259 = throttled 2.0GHz) and sleep ~60-90s between runs. At least one
earlier "regression" diagnosis (the +43ns-per-mm full-width w AP slice
claim below) is suspect for this reason.

Things measured NOT to help (kept out): gpsimd tensor_add in the tail
normalize (ucode pool-config latency + 1.15us/op), 256/128-col final
sub-tiles, interleaved 256-col half-chains for the final epilogue
(260us clean - extra readaccs/instruction overhead beat the chain
shortening), k-outer interleave of the first two phase-1 windows,
moving x2 to the gpsimd ring, all-bf16 tail stt (DVE stt measures the
SAME 743ns for bf16 as f32 - no 2x 16-bit mode for this op), SWDGE
accum_op=add output stores with a DRAM->DRAM f32 bias pre-store (tail
compute -2us but the accum transfers drag the last DMA from 252 to
258us), 1024-col batched normalize stt+stores (neutral). The head is
DMA-bandwidth-bound, not pacing-bound: cumulative delivery measures
300-350GB/s from the first packet, and window j=0's 3.6MB floor matches
the observed stream start + early stalls within ~1.5us. The ~10us
teardown is the TileContext global-clock drain + per-sem barrier parade
(~57 EVENT_SEMAPHOREs per engine at ~115ns) - framework-fixed.
Run-to-run HW variance is +/-2us on a clean clock.
"""

import sys

if "/opt/trn_rl_repo" not in sys.path:
    sys.path.insert(0, "/opt/trn_rl_repo")

import ml_dtypes  # noqa: F401  (np bf16/fp8 dtypes)
import numpy as np

import concourse.bass as bass  # noqa: F401
import concourse.tile as tile
from concourse import bacc, mybir
from concourse.bass_utils import run_bass_kernel_spmd

P = 128
GELU_A = 0.044715
GELU_C = 0.7978845608

FULL_M, FULL_K, FULL_N = 8192, 4096, 4096
NCORES = 8
MC = FULL_M // NCORES   # rows per core
NT = 512                # n tile (columns per psum tile)
PH1 = 3                 # m-tiles computed j-outer while W streams in

W_SCALE = 64.0  # weight ~U(-1/64,1/64) sits at e4m3's min-normal boundary;
                # scale into [-1,1] for the matmul, undo in the epilogue.


def build_nc(MC=MC, K=FULL_K, N=FULL_N, NT=NT, ph1=PH1):
    """Emit the per-core Bass program. Each core computes MC rows."""
    KO = K // P            # 32 k-subtiles of 128
    MT = MC // P           # 8 m-tiles of 128 rows
    NTILES = N // NT       # 8 n-slabs
    f32 = mybir.dt.float32
    bf16 = mybir.dt.bfloat16
    fp8 = mybir.dt.float8e4
    inv_s = 1.0 / W_SCALE

    nc = bacc.Bacc("TRN2", target_bir_lowering=False, debug=False)
    xt = nc.dram_tensor("xt", [MT, P, KO, P], fp8, kind="ExternalInput").ap()
    wt = nc.dram_tensor("wt", [NTILES, P, KO, NT], fp8, kind="ExternalInput").ap()
    bias = nc.dram_tensor("bias", [P, N], bf16, kind="ExternalInput").ap()
    out = nc.dram_tensor("out", [P, MT, N], f32, kind="ExternalOutput").ap()

    with tile.TileContext(nc) as tc:
        WCH = 4            # k-chunks per W slab DMA (matmuls start on chunk 0)
        KW = KO // WCH
        with (
            tc.tile_pool(name="const", bufs=1) as const_pool,
            tc.tile_pool(name="x", bufs=4) as x_pool,
            tc.tile_pool(name="probs", bufs=4) as probs_pool,
            tc.tile_pool(name="tmp", bufs=2) as tmp_pool,
            tc.tile_pool(name="stage", bufs=6) as stage_pool,
            tc.tile_pool(name="psum", bufs=8, space="PSUM") as psum_pool,
        ):
            w_all = const_pool.tile([P, NTILES, KO, NT], fp8, tag="w")
            bias_t = const_pool.tile([P, N], bf16, tag="bias")
            sums = const_pool.tile([P, MT * NTILES + 2], f32, tag="sums")
            ssum = const_pool.tile([P, MT], f32, tag="ssum")
            recips = const_pool.tile([P, MT], f32, tag="recips")

            xts = {}

            def load_x(i, eng=None, nsplit=1):
                # x issues ride the SP queue so they run in parallel with the
                # W issues on GpSimd (separate descriptor streams).
                xts[i] = x_pool.tile([P, KO, P], fp8, tag="xc", name=f"x{i}")
                e = eng or nc.sync
                kw = KO // nsplit
                for c in range(nsplit):
                    e.dma_start(
                        xts[i][:, c * kw : (c + 1) * kw, :],
                        xt[i, :, c * kw : (c + 1) * kw, :],
                    )

            def load_w_slab(j, splits=None):
                ko_edges = splits or [KW * c for c in range(WCH)] + [KO]
                for c in range(len(ko_edges) - 1):
                    lo, hi = ko_edges[c], ko_edges[c + 1]
                    nc.gpsimd.dma_start(
                        w_all[:, j, lo:hi, :], wt[j, :, lo:hi, :]
                    )

            # Head: x0 + first W slab first (critical path of matmul 0), then
            # the rest of W in consumption order. Slab 0 uses fine k-chunks so
            # the first matmuls start as soon as ~0.13MB lands.
            load_x(0, nsplit=4)
            load_w_slab(0, splits=[0, 2, 4, 8, 16, 24, 32])
            load_x(1)
            load_x(2)
            load_w_slab(1)
            load_w_slab(2)
            nc.gpsimd.dma_start(bias_t[:], bias[:])
            for j in range(3, NTILES):
                load_w_slab(j)
            load_x(ph1)  # prefetch into the 4th x slot during phase 1

            probs = {}

            def mm_psum(i, j):
                """16 DoubleRow matmuls accumulating one 128x512 tile."""
                ps = psum_pool.tile([P, NT], f32, name="ps", tag="ps")
                xti = xts[i]
                for k in range(0, KO, 2):
                    nc.tensor.matmul(
                        ps[:],
                        xti[:, k : k + 2, :],
                        w_all[:, j, k : k + 2, :],
                        start=(k == 0),
                        stop=(k == KO - 2),
                        perf_mode=mybir.MatmulPerfMode.DoubleRow,
                    )
                return ps

            def epilogue(i, lo, hi, sum_slot, ps, off=0):
                """p = exp(gelu(v)) for ps[:, off:off+(hi-lo)], ps = W_SCALE*v.
                Square/Tanh/Exp all live in the exp_and_others ACT table.
                Each op reads PSUM at most once."""
                nt = hi - lo
                pss = ps[:, off : off + nt] if nt != NT else ps[:]
                v2 = tmp_pool.tile([P, nt], f32, tag="v2", bufs=1, name="v2")
                nc.scalar.activation(
                    v2[:], pss, mybir.ActivationFunctionType.Square,
                    bias=0.0, scale=float(np.sqrt(GELU_A) * inv_s),
                )
                t2 = tmp_pool.tile([P, nt], f32, tag="t2", bufs=1, name="t2")
                nc.vector.scalar_tensor_tensor(
                    t2[:], v2[:], 1.0, pss,
                    mybir.AluOpType.add, mybir.AluOpType.mult,
                )
                th = tmp_pool.tile([P, nt], f32, tag="th", bufs=1, name="th")
                nc.scalar.activation(
                    th[:], t2[:], mybir.ActivationFunctionType.Tanh,
                    bias=0.0, scale=GELU_C * inv_s,
                )
                g2 = tmp_pool.tile([P, nt], f32, tag="g2", name="g2")
                nc.vector.scalar_tensor_tensor(
                    g2[:], th[:], 1.0, pss,
                    mybir.AluOpType.add, mybir.AluOpType.mult,
                )
                nc.scalar.activation(
                    probs[i][:, lo:hi], g2[:],
                    mybir.ActivationFunctionType.Exp,
                    bias=0.0, scale=0.5 * inv_s,
                    accum_out=sum_slot,
                )

            def mm_tile(i, lo, hi, sum_slot):
                assert hi - lo == NT and lo % NT == 0
                ps = mm_psum(i, lo // NT)
                epilogue(i, lo, hi, sum_slot, ps)

            def normalize(i, js, eng):
                """probs[i] * 1/rowsum + bias -> out, for n-slabs js."""
                for j in js:
                    st = stage_pool.tile([P, NT], f32)
                    eng.scalar_tensor_tensor(
                        st[:],
                        probs[i][:, j * NT : (j + 1) * NT],
                        recips[:, i : i + 1],
                        bias_t[:, j * NT : (j + 1) * NT],
                        mybir.AluOpType.mult,
                        mybir.AluOpType.add,
                    )
                    nc.sync.dma_start(out[:, i, j * NT : (j + 1) * NT], st[:])

            def row_stats(i, nslots=NTILES):
                nc.vector.reduce_sum(
                    ssum[:, i : i + 1],
                    sums[:, i * NTILES : i * NTILES + nslots],
                    axis=mybir.AxisListType.X,
                )
                nc.vector.reciprocal(recips[:, i : i + 1], ssum[:, i : i + 1])

            ALLJ = range(NTILES)

            def slot(s):
                return sums[:, s : s + 1]

            # Phase 1: j-outer so each W slab is used for all ph1 m-tiles as
            # soon as it lands.
            for i in range(ph1):
                probs[i] = probs_pool.tile([P, N], bf16, tag="probs", name=f"probs{i}")
            for j in ALLJ:
                for i in range(ph1):
                    mm_tile(i, j * NT, (j + 1) * NT, slot(i * NTILES + j))


            # Phase 2: i-outer; normalize of tile i overlaps later matmuls.
            # Emission order is load-bearing two ways: norm(3)/norm(0) must
            # head the DVE queue at window i=4 so probs slots A/D recycle in
            # time, while the remaining norms are spread mid-j-loop across
            # windows i=4..7 so no window's DVE queue (epilogue + norms)
            # overflows its 27.6us matmul budget.
            norm_at = {(4, 3): 1, (5, 1): 2, (5, 5): 4, (6, 3): 5, (7, 3): 6}
            for i in range(ph1, MT):
                probs[i] = probs_pool.tile([P, N], bf16, tag="probs", name=f"probs{i}")
                if i == ph1:
                    load_x(i + 1)  # slots freed when phase 1 released x0..x2
                if i + 2 <= MT - 1:
                    load_x(i + 2)
                base = i * NTILES
                for j in ALLJ:
                    mm_tile(i, j * NT, (j + 1) * NT, slot(base + j))
                    if (i, j) in norm_at:
                        normalize(norm_at[(i, j)], ALLJ, nc.vector)
                row_stats(i)
                if i == ph1:
                    normalize(i, ALLJ, nc.vector)
                    for b in range(ph1):
                        row_stats(b)
                    normalize(0, ALLJ, nc.vector)
            normalize(MT - 1, ALLJ, nc.vector)
    nc.compile()
    return nc


def pack_inputs(x, weight, bias, MC=MC, NT=NT):
    """Host-side shard + pack into the DMA-friendly layouts the kernel expects."""
    M, K = x.shape
    N = weight.shape[0]
    KO = K // P
    MT = MC // P
    NTILES = N // NT
    ncores = M // MC
    fp8 = ml_dtypes.float8_e4m3fn
    # wt[j, p, ko, n] = 64*weight[j*NT+n, ko*P+p]
    wt = np.ascontiguousarray(
        (weight * W_SCALE).astype(fp8).reshape(NTILES, NT, KO, P).transpose(0, 3, 2, 1)
    )
    bias_b = np.ascontiguousarray(
        np.broadcast_to(bias.astype(ml_dtypes.bfloat16)[None, :], (P, N))
    )
    in_maps = []
    for c in range(ncores):
        xs = x[c * MC : (c + 1) * MC].astype(fp8)
        # xt[i, p, ko, m] = x_core[i*P+m, ko*P+p]
        xtc = np.ascontiguousarray(xs.reshape(MT, P, KO, P).transpose(0, 3, 2, 1))
        in_maps.append({"xt": xtc, "wt": wt, "bias": bias_b})
    return in_maps


def unpack_outputs(results, MC=MC, N=FULL_N):
    outs = []
    for res in results:
        o = np.asarray(res["out"])  # [P, MT, N]
        outs.append(o.transpose(1, 0, 2).reshape(MC, N))
    return np.concatenate(outs, axis=0)


_CACHE = {}


def _get_nc():
    if "nc" not in _CACHE:
        _CACHE["nc"] = build_nc()
    return _CACHE["nc"]


def _ensure_trace_env():
    """The agent image's antenv lacks axon_hooks, so NTFF tracing silently
    degrades. Register the ctypes-based hook ourselves, and neuter the S3
    artifact upload (no bucket access here)."""
    try:
        from antenv.axon_hooks import get_axon_ntff_profile_hook  # noqa: F401
    except ImportError:
        import types

        import antenv
        from trn_agent_boot.trn_boot import _ntff_profile_via_ctypes

        mod = types.ModuleType("antenv.axon_hooks")
        state = {"hook": _ntff_profile_via_ctypes("/opt/axon/libaxon_pjrt.so")}
        mod.set_axon_ntff_profile_hook = lambda h: state.__setitem__("hook", h)
        mod.get_axon_ntff_profile_hook = lambda: state["hook"]
        sys.modules["antenv.axon_hooks"] = mod
        antenv.axon_hooks = mod
    import concourse.bass_utils as bu

    bu.upload_artifacts = lambda tmpdir: f"local://{tmpdir}"


def kernel(x, weight, bias, trace=False, fp8=True):
    if trace:
        _ensure_trace_env()
    nc = _get_nc()
    in_maps = pack_inputs(
        np.asarray(x, dtype=np.float32),
        np.asarray(weight, dtype=np.float32),
        np.asarray(bias, dtype=np.float32),
    )
    res = run_bass_kernel_spmd(nc, in_maps, core_ids=list(range(NCORES)), trace=trace)
    out = unpack_outputs(res.results)
    if trace:
        return out, res
    return out


# revision 55
# speedup vs baseline: 1.0030x; 1.0030x over previous
"""Trainium2 Bass kernel: out = softmax(gelu_tanh(x @ W^T), axis=-1) + bias.

Full shapes: x [8192, 4096] f32, weight [4096, 4096] f32, bias [4096] f32.
Sharding: data-parallel over rows of x across 8 NeuronCores (1024 rows/core);
weight and bias replicated.

Matmul runs in fp8e4m3 DoubleRow (2 fp8 MACs/cell/cycle = 157 TF/s peak;
measured stream spacing 216ns per 512-col 256-k matmul = 155 TF/s). Weight
values ~U(-1/64,1/64) sit at e4m3's min-normal boundary, so they are
pre-scaled x64 into [-1,1]; the scale is undone inside the fused epilogue.
End-to-end scale-relative error vs the f32 reference is ~1.2e-2 (gate 2e-2):
fp8 quantization of both operands gives ~5% rms per-term error which largely
cancels through the row softmax.

Structure per core (MC=1024 rows = 8 m-tiles of 128):
  - W [4096,4096] fp8 is kept FULLY SBUF-resident (131KB/partition),
    DMA'd once in 512-col slabs.
  - Phase 1 (m-tiles 0..2): j-outer over n-slabs, consuming each W slab as
    it lands (needs ~200GB/s of the 358GB/s DMA peak).
  - Phase 2 (m-tiles 3..7): i-outer — all 4096 columns of one m-tile are
    computed back-to-back, so its softmax row-sum completes immediately and
    the normalize + output DMA overlap the next m-tile's matmuls. Only the
    last m-tile's normalize (~6us on DVE) remains in the tail.
  - Epilogue per 128x512 psum tile: exp(gelu(v)) via ACT Square/Tanh/Exp
    (all share the exp_and_others table -> single ACT_TABLE_LOAD) and two
    DVE scalar_tensor_tensor ops; row sums accumulate via ACT accum_out.
    gelu = 0.5*v*(1+tanh(0.7978845608*(v+0.044715*v^3))) exactly as the
    reference.
  - x loads and output stores issue from the idle SP queue so they overlap
    the W issues on GpSimd; phase-1 normalize backlog drains on DVE during
    early phase-2 windows.

Measured on trn2 (8 cores): ~255-259us HW exec (best 254.9), vs 221.2us
fp8-DoubleRow matmul roofline per core; matmul stream measures 224us with
~6us DMA-ramp stalls at the head, plus ~6.5us fixed engine preamble, ~4us
ACT drain + ~7us normalize at the tail, and ~10us of framework semaphore
teardown. Scale-relative error 1.23e-2 (bf16 bias adds ~6e-4 over the
1.17e-2 of the fp8 matmul). Previous bf16 baseline: 490us; first fp8
version: 302us.

MEASUREMENT TRAP - thermal throttle: back-to-back runs can put the chip
in the P0 power state where EVERY engine runs ~1.2x slower (mm spacing
259ns instead of 216, stt 826 instead of 688) and the kernel measures
~303-305us instead of ~256. ALWAYS validate a measurement by checking
the modal matmul start-to-start spacing in the NTFF (216 = clean 2.4GHz,
259 = throttled 2.0GHz) and sleep ~60-90s between runs. At least one
earlier "regression" diagnosis (the +43ns-per-mm full-width w AP slice
claim below) is suspect for this reason.

Things measured NOT to help (kept out): gpsimd tensor_add in the tail
normalize (ucode pool-config latency + 1.15us/op), 256/128-col final
sub-tiles, interleaved 256-col half-chains for the final epilogue
(260us clean - extra readaccs/instruction overhead beat the chain
shortening), k-outer interleave of the first two phase-1 windows,
moving x2 to the gpsimd ring, all-bf16 tail stt (DVE stt measures the
SAME 743ns for bf16 as f32 - no 2x 16-bit mode for this op), SWDGE
accum_op=add output stores with a DRAM->DRAM f32 bias pre-store (tail
compute -2us but the accum transfers drag the last DMA from 252 to
258us), 1024-col batched normalize stt+stores (neutral). The head is
DMA-bandwidth-bound, not pacing-bound: cumulative delivery measures
300-350GB/s from the first packet, and window j=0's 3.6MB floor matches
the observed stream start + early stalls within ~1.5us. The ~10us
teardown is the TileContext global-clock drain + per-sem barrier parade
(~57 EVENT_SEMAPHOREs per engine at ~115ns) - framework-fixed.
Run-to-run HW variance is +/-2us on a clean clock.
"""

import sys

if "/opt/trn_rl_repo" not in sys.path:
    sys.path.insert(0, "/opt/trn_rl_repo")

import ml_dtypes  # noqa: F401  (np bf16/fp8 dtypes)
import numpy as np

import concourse.bass as bass  # noqa: F401
import concourse.tile as tile
from concourse import bacc, mybir
from concourse.bass_utils import run_bass_kernel_spmd

P = 128
GELU_A = 0.044715
GELU_C = 0.7978845608

FULL_M, FULL_K, FULL_N = 8192, 4096, 4096
NCORES = 8
MC = FULL_M // NCORES   # rows per core
NT = 512                # n tile (columns per psum tile)
PH1 = 3                 # m-tiles computed j-outer while W streams in

W_SCALE = 64.0  # weight ~U(-1/64,1/64) sits at e4m3's min-normal boundary;
                # scale into [-1,1] for the matmul, undo in the epilogue.


def build_nc(MC=MC, K=FULL_K, N=FULL_N, NT=NT, ph1=PH1):
    """Emit the per-core Bass program. Each core computes MC rows."""
    KO = K // P            # 32 k-subtiles of 128
    MT = MC // P           # 8 m-tiles of 128 rows
    NTILES = N // NT       # 8 n-slabs
    f32 = mybir.dt.float32
    bf16 = mybir.dt.bfloat16
    fp8 = mybir.dt.float8e4
    inv_s = 1.0 / W_SCALE

    nc = bacc.Bacc("TRN2", target_bir_lowering=False, debug=False)
    xt = nc.dram_tensor("xt", [MT, P, KO, P], fp8, kind="ExternalInput").ap()
    wt = nc.dram_tensor("wt", [NTILES, P, KO, NT], fp8, kind="ExternalInput").ap()
    bias = nc.dram_tensor("bias", [P, N], bf16, kind="ExternalInput").ap()
    out = nc.dram_tensor("out", [P, MT, N], f32, kind="ExternalOutput").ap()

    with tile.TileContext(nc) as tc:
        WCH = 4            # k-chunks per W slab DMA (matmuls start on chunk 0)
        KW = KO // WCH
        with (
            tc.tile_pool(name="const", bufs=1) as const_pool,
            tc.tile_pool(name="x", bufs=4) as x_pool,
            tc.tile_pool(name="probs", bufs=4) as probs_pool,
            tc.tile_pool(name="tmp", bufs=2) as tmp_pool,
            tc.tile_pool(name="stage", bufs=6) as stage_pool,
            tc.tile_pool(name="psum", bufs=8, space="PSUM") as psum_pool,
        ):
            w_all = const_pool.tile([P, NTILES, KO, NT], fp8, tag="w")
            bias_t = const_pool.tile([P, N], bf16, tag="bias")
            sums = const_pool.tile([P, MT * NTILES + 2], f32, tag="sums")
            ssum = const_pool.tile([P, MT], f32, tag="ssum")
            recips = const_pool.tile([P, MT], f32, tag="recips")

            xts = {}

            def load_x(i, eng=None, nsplit=1):
                # x issues ride the SP queue so they run in parallel with the
                # W issues on GpSimd (separate descriptor streams).
                xts[i] = x_pool.tile([P, KO, P], fp8, tag="xc", name=f"x{i}")
                e = eng or nc.sync
                kw = KO // nsplit
                for c in range(nsplit):
                    e.dma_start(
                        xts[i][:, c * kw : (c + 1) * kw, :],
                        xt[i, :, c * kw : (c + 1) * kw, :],
                    )

            def load_w_slab(j, splits=None):
                ko_edges = splits or [KW * c for c in range(WCH)] + [KO]
                for c in range(len(ko_edges) - 1):
                    lo, hi = ko_edges[c], ko_edges[c + 1]
                    nc.gpsimd.dma_start(
                        w_all[:, j, lo:hi, :], wt[j, :, lo:hi, :]
                    )

            # Head: x0 + first W slab first (critical path of matmul 0), then
            # the rest of W in consumption order. Slab 0 uses fine k-chunks so
            # the first matmuls start as soon as ~0.13MB lands.
            load_x(0, nsplit=4)
            load_w_slab(0, splits=[0, 2, 4, 8, 16, 24, 32])
            load_x(1)
            load_x(2)
            load_w_slab(1, splits=[0, 4, 8, 16, 24, 32])
            load_w_slab(2)
            nc.gpsimd.dma_start(bias_t[:], bias[:])
            for j in range(3, NTILES):
                load_w_slab(j)
            load_x(ph1)  # prefetch into the 4th x slot during phase 1

            probs = {}

            def mm_psum(i, j):
                """16 DoubleRow matmuls accumulating one 128x512 tile."""
                ps = psum_pool.tile([P, NT], f32, name="ps", tag="ps")
                xti = xts[i]
                for k in range(0, KO, 2):
                    nc.tensor.matmul(
                        ps[:],
                        xti[:, k : k + 2, :],
                        w_all[:, j, k : k + 2, :],
                        start=(k == 0),
                        stop=(k == KO - 2),
                        perf_mode=mybir.MatmulPerfMode.DoubleRow,
                    )
                return ps

            def epilogue(i, lo, hi, sum_slot, ps, off=0):
                """p = exp(gelu(v)) for ps[:, off:off+(hi-lo)], ps = W_SCALE*v.
                Square/Tanh/Exp all live in the exp_and_others ACT table.
                Each op reads PSUM at most once."""
                nt = hi - lo
                pss = ps[:, off : off + nt] if nt != NT else ps[:]
                v2 = tmp_pool.tile([P, nt], f32, tag="v2", bufs=1, name="v2")
                nc.scalar.activation(
                    v2[:], pss, mybir.ActivationFunctionType.Square,
                    bias=0.0, scale=float(np.sqrt(GELU_A) * inv_s),
                )
                t2 = tmp_pool.tile([P, nt], f32, tag="t2", bufs=1, name="t2")
                nc.vector.scalar_tensor_tensor(
                    t2[:], v2[:], 1.0, pss,
                    mybir.AluOpType.add, mybir.AluOpType.mult,
                )
                th = tmp_pool.tile([P, nt], f32, tag="th", bufs=1, name="th")
                nc.scalar.activation(
                    th[:], t2[:], mybir.ActivationFunctionType.Tanh,
                    bias=0.0, scale=GELU_C * inv_s,
                )
                g2 = tmp_pool.tile([P, nt], f32, tag="g2", name="g2")
                nc.vector.scalar_tensor_tensor(
                    g2[:], th[:], 1.0, pss,
                    mybir.AluOpType.add, mybir.AluOpType.mult,
                )
                nc.scalar.activation(
                    probs[i][:, lo:hi], g2[:],
                    mybir.ActivationFunctionType.Exp,
                    bias=0.0, scale=0.5 * inv_s,
                    accum_out=sum_slot,
                )

            def mm_tile(i, lo, hi, sum_slot):
                assert hi - lo == NT and lo % NT == 0
                ps = mm_psum(i, lo // NT)
                epilogue(i, lo, hi, sum_slot, ps)

            def normalize(i, js, eng):
                """probs[i] * 1/rowsum + bias -> out, for n-slabs js."""
                for j in js:
                    st = stage_pool.tile([P, NT], f32)
                    eng.scalar_tensor_tensor(
                        st[:],
                        probs[i][:, j * NT : (j + 1) * NT],
                        recips[:, i : i + 1],
                        bias_t[:, j * NT : (j + 1) * NT],
                        mybir.AluOpType.mult,
                        mybir.AluOpType.add,
                    )
                    nc.sync.dma_start(out[:, i, j * NT : (j + 1) * NT], st[:])

            def row_stats(i, nslots=NTILES):
                nc.vector.reduce_sum(
                    ssum[:, i : i + 1],
                    sums[:, i * NTILES : i * NTILES + nslots],
                    axis=mybir.AxisListType.X,
                )
                nc.vector.reciprocal(recips[:, i : i + 1], ssum[:, i : i + 1])

            ALLJ = range(NTILES)

            def slot(s):
                return sums[:, s : s + 1]

            # Phase 1: j-outer so each W slab is used for all ph1 m-tiles as
            # soon as it lands.
            for i in range(ph1):
                probs[i] = probs_pool.tile([P, N], bf16, tag="probs", name=f"probs{i}")
            for j in ALLJ:
                for i in range(ph1):
                    mm_tile(i, j * NT, (j + 1) * NT, slot(i * NTILES + j))


            # Phase 2: i-outer; normalize of tile i overlaps later matmuls.
            # Emission order is load-bearing two ways: norm(3)/norm(0) must
            # head the DVE queue at window i=4 so probs slots A/D recycle in
            # time, while the remaining norms are spread mid-j-loop across
            # windows i=4..7 so no window's DVE queue (epilogue + norms)
            # overflows its 27.6us matmul budget.
            norm_at = {(4, 3): 1, (5, 1): 2, (5, 5): 4, (6, 3): 5, (7, 3): 6}
            for i in range(ph1, MT):
                probs[i] = probs_pool.tile([P, N], bf16, tag="probs", name=f"probs{i}")
                if i == ph1:
                    load_x(i + 1)  # slots freed when phase 1 released x0..x2
                if i + 2 <= MT - 1:
                    load_x(i + 2)
                base = i * NTILES
                for j in ALLJ:
                    mm_tile(i, j * NT, (j + 1) * NT, slot(base + j))
                    if (i, j) in norm_at:
                        normalize(norm_at[(i, j)], ALLJ, nc.vector)
                row_stats(i)
                if i == ph1:
                    normalize(i, ALLJ, nc.vector)
                    for b in range(ph1):
                        row_stats(b)
                    normalize(0, ALLJ, nc.vector)
            normalize(MT - 1, ALLJ, nc.vector)
    nc.compile()
    return nc


def pack_inputs(x, weight, bias, MC=MC, NT=NT):
    """Host-side shard + pack into the DMA-friendly layouts the kernel expects."""
    M, K = x.shape
    N = weight.shape[0]
    KO = K // P
    MT = MC // P
    NTILES = N // NT
    ncores = M // MC
    fp8 = ml_dtypes.float8_e4m3fn
    # wt[j, p, ko, n] = 64*weight[j*NT+n, ko*P+p]
    wt = np.ascontiguousarray(
        (weight * W_SCALE).astype(fp8).reshape(NTILES, NT, KO, P).transpose(0, 3, 2, 1)
    )
    bias_b = np.ascontiguousarray(
        np.broadcast_to(bias.astype(ml_dtypes.bfloat16)[None, :], (P, N))
    )
    in_maps = []
    for c in range(ncores):
        xs = x[c * MC : (c + 1) * MC].astype(fp8)
        # xt[i, p, ko, m] = x_core[i*P+m, ko*P+p]
        xtc = np.ascontiguousarray(xs.reshape(MT, P, KO, P).transpose(0, 3, 2, 1))
        in_maps.append({"xt": xtc, "wt": wt, "bias": bias_b})
    return in_maps


def unpack_outputs(results, MC=MC, N=FULL_N):
    outs = []
    for res in results:
        o = np.asarray(res["out"])  # [P, MT, N]
        outs.append(o.transpose(1, 0, 2).reshape(MC, N))
    return np.concatenate(outs, axis=0)


_CACHE = {}


def _get_nc():
    if "nc" not in _CACHE:
        _CACHE["nc"] = build_nc()
    return _CACHE["nc"]


def _ensure_trace_env():
    """The agent image's antenv lacks axon_hooks, so NTFF tracing silently
    degrades. Register the ctypes-based hook ourselves, and neuter the S3
    artifact upload (no bucket access here)."""
    try:
        from antenv.axon_hooks import get_axon_ntff_profile_hook  # noqa: F401
    except ImportError:
        import types

        import antenv
        from trn_agent_boot.trn_boot import _ntff_profile_via_ctypes

        mod = types.ModuleType("antenv.axon_hooks")
        state = {"hook": _ntff_profile_via_ctypes("/opt/axon/libaxon_pjrt.so")}
        mod.set_axon_ntff_profile_hook = lambda h: state.__setitem__("hook", h)
        mod.get_axon_ntff_profile_hook = lambda: state["hook"]
        sys.modules["antenv.axon_hooks"] = mod
        antenv.axon_hooks = mod
    import concourse.bass_utils as bu

    bu.upload_artifacts = lambda tmpdir: f"local://{tmpdir}"


def kernel(x, weight, bias, trace=False, fp8=True):
    if trace:
        _ensure_trace_env()
    nc = _get_nc()
    in_maps = pack_inputs(
        np.asarray(x, dtype=np.float32),
        np.asarray(weight, dtype=np.float32),
        np.asarray(bias, dtype=np.float32),
    )
    res = run_bass_kernel_spmd(nc, in_maps, core_ids=list(range(NCORES)), trace=trace)
    out = unpack_outputs(res.results)
    if trace:
        return out, res
    return out


# revision 57
# speedup vs baseline: 1.0097x; 1.0067x over previous
"""Trainium2 Bass kernel: out = softmax(gelu_tanh(x @ W^T), axis=-1) + bias.

Full shapes: x [8192, 4096] f32, weight [4096, 4096] f32, bias [4096] f32.
Sharding: data-parallel over rows of x across 8 NeuronCores (1024 rows/core);
weight and bias replicated.

Matmul runs in fp8e4m3 DoubleRow (2 fp8 MACs/cell/cycle = 157 TF/s peak;
measured stream spacing 216ns per 512-col 256-k matmul = 155 TF/s). Weight
values ~U(-1/64,1/64) sit at e4m3's min-normal boundary, so they are
pre-scaled x64 into [-1,1]; the scale is undone inside the fused epilogue.
End-to-end scale-relative error vs the f32 reference is ~1.2e-2 (gate 2e-2):
fp8 quantization of both operands gives ~5% rms per-term error which largely
cancels through the row softmax.

Structure per core (MC=1024 rows = 8 m-tiles of 128):
  - W [4096,4096] fp8 is kept FULLY SBUF-resident (131KB/partition),
    DMA'd once in 512-col slabs.
  - Phase 1 (m-tiles 0..2): j-outer over n-slabs, consuming each W slab as
    it lands (needs ~200GB/s of the 358GB/s DMA peak).
  - Phase 2 (m-tiles 3..7): i-outer — all 4096 columns of one m-tile are
    computed back-to-back, so its softmax row-sum completes immediately and
    the normalize + output DMA overlap the next m-tile's matmuls. Only the
    last m-tile's normalize (~6us on DVE) remains in the tail.
  - Epilogue per 128x512 psum tile: exp(gelu(v)) via ACT Square/Tanh/Exp
    (all share the exp_and_others table -> single ACT_TABLE_LOAD) and two
    DVE scalar_tensor_tensor ops; row sums accumulate via ACT accum_out.
    gelu = 0.5*v*(1+tanh(0.7978845608*(v+0.044715*v^3))) exactly as the
    reference.
  - x loads and output stores issue from the idle SP queue so they overlap
    the W issues on GpSimd; phase-1 normalize backlog drains on DVE during
    early phase-2 windows.

Measured on trn2 (8 cores): 255-259us HW exec on a clean clock (mean
~257, best 254.9), vs 221.2us fp8-DoubleRow matmul roofline per core;
matmul stream measures ~224us with ~6us DMA-ramp stalls at the head
(bandwidth-bound, see below), plus ~6.5us fixed engine preamble, ~4us
ACT drain + ~7us normalize at the tail, and ~10us of framework
semaphore teardown. Scale-relative error 1.23e-2 (bf16 bias adds ~6e-4
over the 1.17e-2 of the fp8 matmul). Previous bf16 baseline: 490us;
first fp8 version: 302us.

MEASUREMENT TRAP - thermal throttle: back-to-back runs can put the chip
in the P0 power state where EVERY engine runs ~1.2x slower (mm spacing
259ns instead of 216, stt 826 instead of 688) and the kernel measures
~303-305us instead of ~256. ALWAYS validate a measurement by checking
the modal matmul start-to-start spacing in the NTFF (216 = clean 2.4GHz,
259 = throttled 2.0GHz) and sleep ~60-90s between runs. At least one
earlier "regression" diagnosis (the +43ns-per-mm full-width w AP slice
claim below) is suspect for this reason.

Things measured NOT to help (kept out): gpsimd tensor_add in the tail
normalize (ucode pool-config latency + 1.15us/op), 256/128-col final
sub-tiles, interleaved 256-col half-chains for the final epilogue
(260us clean - extra readaccs/instruction overhead beat the chain
shortening), k-outer interleave of the first two phase-1 windows,
moving x2 to the gpsimd ring, all-bf16 tail stt (DVE stt measures the
SAME 743ns for bf16 as f32 - no 2x 16-bit mode for this op), SWDGE
accum_op=add output stores with a DRAM->DRAM f32 bias pre-store (tail
compute -2us but the accum transfers drag the last DMA from 252 to
258us), 1024-col batched normalize stt+stores (neutral). The head is
DMA-bandwidth-bound, not pacing-bound: cumulative delivery measures
300-350GB/s from the first packet, and window j=0's 3.6MB floor matches
the observed stream start + early stalls within ~1.5us. The ~10us
teardown is the TileContext global-clock drain + per-sem barrier parade
(~57 EVENT_SEMAPHOREs per engine at ~115ns) - framework-fixed.
Run-to-run HW variance is +/-2us on a clean clock.
"""

import sys

if "/opt/trn_rl_repo" not in sys.path:
    sys.path.insert(0, "/opt/trn_rl_repo")

import ml_dtypes  # noqa: F401  (np bf16/fp8 dtypes)
import numpy as np

import concourse.bass as bass  # noqa: F401
import concourse.tile as tile
from concourse import bacc, mybir
from concourse.bass_utils import run_bass_kernel_spmd

P = 128
GELU_A = 0.044715
GELU_C = 0.7978845608

FULL_M, FULL_K, FULL_N = 8192, 4096, 4096
NCORES = 8
MC = FULL_M // NCORES   # rows per core
NT = 512                # n tile (columns per psum tile)
PH1 = 3                 # m-tiles computed j-outer while W streams in

W_SCALE = 64.0  # weight ~U(-1/64,1/64) sits at e4m3's min-normal boundary;
                # scale into [-1,1] for the matmul, undo in the epilogue.


def build_nc(MC=MC, K=FULL_K, N=FULL_N, NT=NT, ph1=PH1):
    """Emit the per-core Bass program. Each core computes MC rows."""
    KO = K // P            # 32 k-subtiles of 128
    MT = MC // P           # 8 m-tiles of 128 rows
    NTILES = N // NT       # 8 n-slabs
    f32 = mybir.dt.float32
    bf16 = mybir.dt.bfloat16
    fp8 = mybir.dt.float8e4
    inv_s = 1.0 / W_SCALE

    nc = bacc.Bacc("TRN2", target_bir_lowering=False, debug=False)
    xt = nc.dram_tensor("xt", [MT, P, KO, P], fp8, kind="ExternalInput").ap()
    wt = nc.dram_tensor("wt", [NTILES, P, KO, NT], fp8, kind="ExternalInput").ap()
    bias = nc.dram_tensor("bias", [P, N], bf16, kind="ExternalInput").ap()
    out = nc.dram_tensor("out", [P, MT, N], f32, kind="ExternalOutput").ap()

    with tile.TileContext(nc) as tc:
        WCH = 4            # k-chunks per W slab DMA (matmuls start on chunk 0)
        KW = KO // WCH
        with (
            tc.tile_pool(name="const", bufs=1) as const_pool,
            tc.tile_pool(name="x", bufs=4) as x_pool,
            tc.tile_pool(name="probs", bufs=4) as probs_pool,
            tc.tile_pool(name="tmp", bufs=2) as tmp_pool,
            tc.tile_pool(name="stage", bufs=6) as stage_pool,
            tc.tile_pool(name="psum", bufs=8, space="PSUM") as psum_pool,
        ):
            w_all = const_pool.tile([P, NTILES, KO, NT], fp8, tag="w")
            bias_t = const_pool.tile([P, N], bf16, tag="bias")
            sums = const_pool.tile([P, MT * NTILES + 2], f32, tag="sums")
            ssum = const_pool.tile([P, MT], f32, tag="ssum")
            recips = const_pool.tile([P, MT], f32, tag="recips")

            xts = {}

            def load_x(i, eng=None, nsplit=1):
                # x issues ride the SP queue so they run in parallel with the
                # W issues on GpSimd (separate descriptor streams).
                xts[i] = x_pool.tile([P, KO, P], fp8, tag="xc", name=f"x{i}")
                e = eng or nc.sync
                kw = KO // nsplit
                for c in range(nsplit):
                    e.dma_start(
                        xts[i][:, c * kw : (c + 1) * kw, :],
                        xt[i, :, c * kw : (c + 1) * kw, :],
                    )

            def load_w_slab(j, splits=None):
                ko_edges = splits or [KW * c for c in range(WCH)] + [KO]
                for c in range(len(ko_edges) - 1):
                    lo, hi = ko_edges[c], ko_edges[c + 1]
                    nc.gpsimd.dma_start(
                        w_all[:, j, lo:hi, :], wt[j, :, lo:hi, :]
                    )

            # Head: x0 + first W slab first (critical path of matmul 0), then
            # the rest of W in consumption order. Slab 0 uses fine k-chunks so
            # the first matmuls start as soon as ~0.13MB lands.
            load_x(0, nsplit=4)
            load_w_slab(0, splits=[0, 2, 4, 8, 16, 24, 32])
            load_x(1)
            load_x(2)
            load_w_slab(1, splits=[0, 4, 8, 16, 24, 32])
            load_w_slab(2)
            nc.gpsimd.dma_start(bias_t[:], bias[:])
            for j in range(3, NTILES):
                # delivery runs ~1.6x ahead of consumption by here; coarse
                # 2-chunk slabs halve the issue count (queue time + sems)
                load_w_slab(j, splits=[0, 16, 32])
            load_x(ph1)  # prefetch into the 4th x slot during phase 1

            probs = {}

            def mm_psum(i, j):
                """16 DoubleRow matmuls accumulating one 128x512 tile."""
                ps = psum_pool.tile([P, NT], f32, name="ps", tag="ps")
                xti = xts[i]
                for k in range(0, KO, 2):
                    nc.tensor.matmul(
                        ps[:],
                        xti[:, k : k + 2, :],
                        w_all[:, j, k : k + 2, :],
                        start=(k == 0),
                        stop=(k == KO - 2),
                        perf_mode=mybir.MatmulPerfMode.DoubleRow,
                    )
                return ps

            def epilogue(i, lo, hi, sum_slot, ps, off=0):
                """p = exp(gelu(v)) for ps[:, off:off+(hi-lo)], ps = W_SCALE*v.
                Square/Tanh/Exp all live in the exp_and_others ACT table.
                Each op reads PSUM at most once."""
                nt = hi - lo
                pss = ps[:, off : off + nt] if nt != NT else ps[:]
                v2 = tmp_pool.tile([P, nt], f32, tag="v2", bufs=1, name="v2")
                nc.scalar.activation(
                    v2[:], pss, mybir.ActivationFunctionType.Square,
                    bias=0.0, scale=float(np.sqrt(GELU_A) * inv_s),
                )
                t2 = tmp_pool.tile([P, nt], f32, tag="t2", bufs=1, name="t2")
                nc.vector.scalar_tensor_tensor(
                    t2[:], v2[:], 1.0, pss,
                    mybir.AluOpType.add, mybir.AluOpType.mult,
                )
                th = tmp_pool.tile([P, nt], f32, tag="th", bufs=1, name="th")
                nc.scalar.activation(
                    th[:], t2[:], mybir.ActivationFunctionType.Tanh,
                    bias=0.0, scale=GELU_C * inv_s,
                )
                g2 = tmp_pool.tile([P, nt], f32, tag="g2", name="g2")
                nc.vector.scalar_tensor_tensor(
                    g2[:], th[:], 1.0, pss,
                    mybir.AluOpType.add, mybir.AluOpType.mult,
                )
                nc.scalar.activation(
                    probs[i][:, lo:hi], g2[:],
                    mybir.ActivationFunctionType.Exp,
                    bias=0.0, scale=0.5 * inv_s,
                    accum_out=sum_slot,
                )

            def mm_tile(i, lo, hi, sum_slot):
                assert hi - lo == NT and lo % NT == 0
                ps = mm_psum(i, lo // NT)
                epilogue(i, lo, hi, sum_slot, ps)

            def normalize(i, js, eng):
                """probs[i] * 1/rowsum + bias -> out, for n-slabs js."""
                for j in js:
                    st = stage_pool.tile([P, NT], f32)
                    eng.scalar_tensor_tensor(
                        st[:],
                        probs[i][:, j * NT : (j + 1) * NT],
                        recips[:, i : i + 1],
                        bias_t[:, j * NT : (j + 1) * NT],
                        mybir.AluOpType.mult,
                        mybir.AluOpType.add,
                    )
                    nc.sync.dma_start(out[:, i, j * NT : (j + 1) * NT], st[:])

            def row_stats(i, nslots=NTILES):
                nc.vector.reduce_sum(
                    ssum[:, i : i + 1],
                    sums[:, i * NTILES : i * NTILES + nslots],
                    axis=mybir.AxisListType.X,
                )
                nc.vector.reciprocal(recips[:, i : i + 1], ssum[:, i : i + 1])

            ALLJ = range(NTILES)

            def slot(s):
                return sums[:, s : s + 1]

            # Phase 1: j-outer so each W slab is used for all ph1 m-tiles as
            # soon as it lands.
            for i in range(ph1):
                probs[i] = probs_pool.tile([P, N], bf16, tag="probs", name=f"probs{i}")
            for j in ALLJ:
                for i in range(ph1):
                    mm_tile(i, j * NT, (j + 1) * NT, slot(i * NTILES + j))


            # Phase 2: i-outer; normalize of tile i overlaps later matmuls.
            # Emission order is load-bearing two ways: norm(3)/norm(0) must
            # head the DVE queue at window i=4 so probs slots A/D recycle in
            # time, while the remaining norms are spread mid-j-loop across
            # windows i=4..7 so no window's DVE queue (epilogue + norms)
            # overflows its 27.6us matmul budget.
            norm_at = {(4, 3): 1, (5, 1): 2, (5, 5): 4, (6, 3): 5, (7, 3): 6}
            for i in range(ph1, MT):
                probs[i] = probs_pool.tile([P, N], bf16, tag="probs", name=f"probs{i}")
                if i == ph1:
                    load_x(i + 1)  # slots freed when phase 1 released x0..x2
                if i + 2 <= MT - 1:
                    load_x(i + 2)
                base = i * NTILES
                for j in ALLJ:
                    mm_tile(i, j * NT, (j + 1) * NT, slot(base + j))
                    if (i, j) in norm_at:
                        normalize(norm_at[(i, j)], ALLJ, nc.vector)
                row_stats(i)
                if i == ph1:
                    normalize(i, ALLJ, nc.vector)
                    for b in range(ph1):
                        row_stats(b)
                    normalize(0, ALLJ, nc.vector)
            normalize(MT - 1, ALLJ, nc.vector)
    nc.compile()
    return nc


def pack_inputs(x, weight, bias, MC=MC, NT=NT):
    """Host-side shard + pack into the DMA-friendly layouts the kernel expects."""
    M, K = x.shape
    N = weight.shape[0]
    KO = K // P
    MT = MC // P
    NTILES = N // NT
    ncores = M // MC
    fp8 = ml_dtypes.float8_e4m3fn
    # wt[j, p, ko, n] = 64*weight[j*NT+n, ko*P+p]
    wt = np.ascontiguousarray(
        (weight * W_SCALE).astype(fp8).reshape(NTILES, NT, KO, P).transpose(0, 3, 2, 1)
    )
    bias_b = np.ascontiguousarray(
        np.broadcast_to(bias.astype(ml_dtypes.bfloat16)[None, :], (P, N))
    )
    in_maps = []
    for c in range(ncores):
        xs = x[c * MC : (c + 1) * MC].astype(fp8)
        # xt[i, p, ko, m] = x_core[i*P+m, ko*P+p]
        xtc = np.ascontiguousarray(xs.reshape(MT, P, KO, P).transpose(0, 3, 2, 1))
        in_maps.append({"xt": xtc, "wt": wt, "bias": bias_b})
    return in_maps


def unpack_outputs(results, MC=MC, N=FULL_N):
    outs = []
    for res in results:
        o = np.asarray(res["out"])  # [P, MT, N]
        outs.append(o.transpose(1, 0, 2).reshape(MC, N))
    return np.concatenate(outs, axis=0)


_CACHE = {}


def _get_nc():
    if "nc" not in _CACHE:
        _CACHE["nc"] = build_nc()
    return _CACHE["nc"]


def _ensure_trace_env():
    """The agent image's antenv lacks axon_hooks, so NTFF tracing silently
    degrades. Register the ctypes-based hook ourselves, and neuter the S3
    artifact upload (no bucket access here)."""
    try:
        from antenv.axon_hooks import get_axon_ntff_profile_hook  # noqa: F401
    except ImportError:
        import types

        import antenv
        from trn_agent_boot.trn_boot import _ntff_profile_via_ctypes

        mod = types.ModuleType("antenv.axon_hooks")
        state = {"hook": _ntff_profile_via_ctypes("/opt/axon/libaxon_pjrt.so")}
        mod.set_axon_ntff_profile_hook = lambda h: state.__setitem__("hook", h)
        mod.get_axon_ntff_profile_hook = lambda: state["hook"]
        sys.modules["antenv.axon_hooks"] = mod
        antenv.axon_hooks = mod
    import concourse.bass_utils as bu

    bu.upload_artifacts = lambda tmpdir: f"local://{tmpdir}"


def kernel(x, weight, bias, trace=False, fp8=True):
    if trace:
        _ensure_trace_env()
    nc = _get_nc()
    in_maps = pack_inputs(
        np.asarray(x, dtype=np.float32),
        np.asarray(weight, dtype=np.float32),
        np.asarray(bias, dtype=np.float32),
    )
    res = run_bass_kernel_spmd(nc, in_maps, core_ids=list(range(NCORES)), trace=trace)
    out = unpack_outputs(res.results)
    if trace:
        return out, res
    return out


# revision 58
# speedup vs baseline: 1.0107x; 1.0009x over previous
"""Trainium2 Bass kernel: out = softmax(gelu_tanh(x @ W^T), axis=-1) + bias.

Full shapes: x [8192, 4096] f32, weight [4096, 4096] f32, bias [4096] f32.
Sharding: data-parallel over rows of x across 8 NeuronCores (1024 rows/core);
weight and bias replicated.

Matmul runs in fp8e4m3 DoubleRow (2 fp8 MACs/cell/cycle = 157 TF/s peak;
measured stream spacing 216ns per 512-col 256-k matmul = 155 TF/s). Weight
values ~U(-1/64,1/64) sit at e4m3's min-normal boundary, so they are
pre-scaled x64 into [-1,1]; the scale is undone inside the fused epilogue.
End-to-end scale-relative error vs the f32 reference is ~1.2e-2 (gate 2e-2):
fp8 quantization of both operands gives ~5% rms per-term error which largely
cancels through the row softmax.

Structure per core (MC=1024 rows = 8 m-tiles of 128):
  - W [4096,4096] fp8 is kept FULLY SBUF-resident (131KB/partition),
    DMA'd once in 512-col slabs.
  - Phase 1 (m-tiles 0..2): j-outer over n-slabs, consuming each W slab as
    it lands (needs ~200GB/s of the 358GB/s DMA peak).
  - Phase 2 (m-tiles 3..7): i-outer — all 4096 columns of one m-tile are
    computed back-to-back, so its softmax row-sum completes immediately and
    the normalize + output DMA overlap the next m-tile's matmuls. Only the
    last m-tile's normalize (~6us on DVE) remains in the tail.
  - Epilogue per 128x512 psum tile: exp(gelu(v)) via ACT Square/Tanh/Exp
    (all share the exp_and_others table -> single ACT_TABLE_LOAD) and two
    DVE scalar_tensor_tensor ops; row sums accumulate via ACT accum_out.
    gelu = 0.5*v*(1+tanh(0.7978845608*(v+0.044715*v^3))) exactly as the
    reference.
  - x loads and output stores issue from the idle SP queue so they overlap
    the W issues on GpSimd; phase-1 normalize backlog drains on DVE during
    early phase-2 windows.

Measured on trn2 (8 cores): 255-259us HW exec on a clean clock (mean
~257, best 254.9), vs 221.2us fp8-DoubleRow matmul roofline per core;
matmul stream measures ~224us with ~6us DMA-ramp stalls at the head
(bandwidth-bound, see below), plus ~6.5us fixed engine preamble, ~4us
ACT drain + ~7us normalize at the tail, and ~10us of framework
semaphore teardown. Scale-relative error 1.23e-2 (bf16 bias adds ~6e-4
over the 1.17e-2 of the fp8 matmul). Previous bf16 baseline: 490us;
first fp8 version: 302us.

MEASUREMENT TRAP - P0 downclock: the device intermittently runs in the
P0 power state where EVERY engine is ~1.2x slower (mm spacing 259ns
instead of 216, stt 826 instead of 688) and the kernel measures
~303-306us instead of ~256. The state flips on a minutes timescale and
is NOT reliably driven by this session's run cadence (observed: clean
after a 5-min sleep, throttled after 15 min idle, two throttled runs
back-to-back, then clean 2 min later) - likely package-level power
management or co-tenant load. ALWAYS classify each measurement by the
modal matmul start-to-start spacing in the NTFF (216 = clean 2.4GHz,
259 = throttled 2.0GHz) and discard/repeat throttled runs; cool-down
sleeps help sample a clean window but guarantee nothing. At least one
earlier "regression" diagnosis (the +43ns-per-mm full-width w AP slice
claim below) is suspect for this reason.

Things measured NOT to help (kept out): gpsimd tensor_add in the tail
normalize (ucode pool-config latency + 1.15us/op), 256/128-col final
sub-tiles, interleaved 256-col half-chains for the final epilogue
(260us clean - extra readaccs/instruction overhead beat the chain
shortening), k-outer interleave of the first two phase-1 windows,
moving x2 to the gpsimd ring, all-bf16 tail stt (DVE stt measures the
SAME 743ns for bf16 as f32 - no 2x 16-bit mode for this op), SWDGE
accum_op=add output stores with a DRAM->DRAM f32 bias pre-store (tail
compute -2us but the accum transfers drag the last DMA from 252 to
258us), 1024-col batched normalize stt+stores (neutral). The head is
DMA-bandwidth-bound, not pacing-bound: cumulative delivery measures
300-350GB/s from the first packet, and window j=0's 3.6MB floor matches
the observed stream start + early stalls within ~1.5us. The ~10us
teardown is the TileContext global-clock drain + per-sem barrier parade
(~57 EVENT_SEMAPHOREs per engine at ~115ns) - framework-fixed.
Run-to-run HW variance is +/-2us on a clean clock.
"""

import sys

if "/opt/trn_rl_repo" not in sys.path:
    sys.path.insert(0, "/opt/trn_rl_repo")

import ml_dtypes  # noqa: F401  (np bf16/fp8 dtypes)
import numpy as np

import concourse.bass as bass  # noqa: F401
import concourse.tile as tile
from concourse import bacc, mybir
from concourse.bass_utils import run_bass_kernel_spmd

P = 128
GELU_A = 0.044715
GELU_C = 0.7978845608

FULL_M, FULL_K, FULL_N = 8192, 4096, 4096
NCORES = 8
MC = FULL_M // NCORES   # rows per core
NT = 512                # n tile (columns per psum tile)
PH1 = 3                 # m-tiles computed j-outer while W streams in

W_SCALE = 64.0  # weight ~U(-1/64,1/64) sits at e4m3's min-normal boundary;
                # scale into [-1,1] for the matmul, undo in the epilogue.


def build_nc(MC=MC, K=FULL_K, N=FULL_N, NT=NT, ph1=PH1):
    """Emit the per-core Bass program. Each core computes MC rows."""
    KO = K // P            # 32 k-subtiles of 128
    MT = MC // P           # 8 m-tiles of 128 rows
    NTILES = N // NT       # 8 n-slabs
    f32 = mybir.dt.float32
    bf16 = mybir.dt.bfloat16
    fp8 = mybir.dt.float8e4
    inv_s = 1.0 / W_SCALE

    nc = bacc.Bacc("TRN2", target_bir_lowering=False, debug=False)
    xt = nc.dram_tensor("xt", [MT, P, KO, P], fp8, kind="ExternalInput").ap()
    wt = nc.dram_tensor("wt", [NTILES, P, KO, NT], fp8, kind="ExternalInput").ap()
    bias = nc.dram_tensor("bias", [P, N], bf16, kind="ExternalInput").ap()
    out = nc.dram_tensor("out", [P, MT, N], f32, kind="ExternalOutput").ap()

    with tile.TileContext(nc) as tc:
        WCH = 4            # k-chunks per W slab DMA (matmuls start on chunk 0)
        KW = KO // WCH
        with (
            tc.tile_pool(name="const", bufs=1) as const_pool,
            tc.tile_pool(name="x", bufs=4) as x_pool,
            tc.tile_pool(name="probs", bufs=4) as probs_pool,
            tc.tile_pool(name="tmp", bufs=2) as tmp_pool,
            tc.tile_pool(name="stage", bufs=6) as stage_pool,
            tc.tile_pool(name="psum", bufs=8, space="PSUM") as psum_pool,
        ):
            w_all = const_pool.tile([P, NTILES, KO, NT], fp8, tag="w")
            bias_t = const_pool.tile([P, N], bf16, tag="bias")
            sums = const_pool.tile([P, MT * NTILES + 2], f32, tag="sums")
            ssum = const_pool.tile([P, MT], f32, tag="ssum")
            recips = const_pool.tile([P, MT], f32, tag="recips")

            xts = {}

            def load_x(i, eng=None, nsplit=1):
                # x issues ride the SP queue so they run in parallel with the
                # W issues on GpSimd (separate descriptor streams).
                xts[i] = x_pool.tile([P, KO, P], fp8, tag="xc", name=f"x{i}")
                e = eng or nc.sync
                kw = KO // nsplit
                for c in range(nsplit):
                    e.dma_start(
                        xts[i][:, c * kw : (c + 1) * kw, :],
                        xt[i, :, c * kw : (c + 1) * kw, :],
                    )

            def load_w_slab(j, splits=None):
                ko_edges = splits or [KW * c for c in range(WCH)] + [KO]
                for c in range(len(ko_edges) - 1):
                    lo, hi = ko_edges[c], ko_edges[c + 1]
                    nc.gpsimd.dma_start(
                        w_all[:, j, lo:hi, :], wt[j, :, lo:hi, :]
                    )

            # Head: x0 + first W slab first (critical path of matmul 0), then
            # the rest of W in consumption order. Slab 0 uses fine k-chunks so
            # the first matmuls start as soon as ~0.13MB lands.
            load_x(0, nsplit=4)
            load_w_slab(0, splits=[0, 2, 4, 8, 16, 24, 32])
            load_x(1)
            load_x(2)
            load_w_slab(1, splits=[0, 4, 8, 16, 24, 32])
            load_w_slab(2)
            nc.gpsimd.dma_start(bias_t[:], bias[:])
            for j in range(3, NTILES):
                # delivery runs ~1.6x ahead of consumption by here; coarse
                # 2-chunk slabs halve the issue count (queue time + sems)
                load_w_slab(j, splits=[0, 16, 32])
            load_x(ph1)  # prefetch into the 4th x slot during phase 1

            probs = {}

            def mm_psum(i, j):
                """16 DoubleRow matmuls accumulating one 128x512 tile."""
                ps = psum_pool.tile([P, NT], f32, name="ps", tag="ps")
                xti = xts[i]
                for k in range(0, KO, 2):
                    nc.tensor.matmul(
                        ps[:],
                        xti[:, k : k + 2, :],
                        w_all[:, j, k : k + 2, :],
                        start=(k == 0),
                        stop=(k == KO - 2),
                        perf_mode=mybir.MatmulPerfMode.DoubleRow,
                    )
                return ps

            def epilogue(i, lo, hi, sum_slot, ps, off=0):
                """p = exp(gelu(v)) for ps[:, off:off+(hi-lo)], ps = W_SCALE*v.
                Square/Tanh/Exp all live in the exp_and_others ACT table.
                Each op reads PSUM at most once."""
                nt = hi - lo
                pss = ps[:, off : off + nt] if nt != NT else ps[:]
                v2 = tmp_pool.tile([P, nt], f32, tag="v2", bufs=1, name="v2")
                nc.scalar.activation(
                    v2[:], pss, mybir.ActivationFunctionType.Square,
                    bias=0.0, scale=float(np.sqrt(GELU_A) * inv_s),
                )
                t2 = tmp_pool.tile([P, nt], f32, tag="t2", bufs=1, name="t2")
                nc.vector.scalar_tensor_tensor(
                    t2[:], v2[:], 1.0, pss,
                    mybir.AluOpType.add, mybir.AluOpType.mult,
                )
                th = tmp_pool.tile([P, nt], f32, tag="th", bufs=1, name="th")
                nc.scalar.activation(
                    th[:], t2[:], mybir.ActivationFunctionType.Tanh,
                    bias=0.0, scale=GELU_C * inv_s,
                )
                g2 = tmp_pool.tile([P, nt], f32, tag="g2", name="g2")
                nc.vector.scalar_tensor_tensor(
                    g2[:], th[:], 1.0, pss,
                    mybir.AluOpType.add, mybir.AluOpType.mult,
                )
                nc.scalar.activation(
                    probs[i][:, lo:hi], g2[:],
                    mybir.ActivationFunctionType.Exp,
                    bias=0.0, scale=0.5 * inv_s,
                    accum_out=sum_slot,
                )

            def mm_tile(i, lo, hi, sum_slot):
                assert hi - lo == NT and lo % NT == 0
                ps = mm_psum(i, lo // NT)
                epilogue(i, lo, hi, sum_slot, ps)

            def normalize(i, js, eng):
                """probs[i] * 1/rowsum + bias -> out, for n-slabs js."""
                for j in js:
                    st = stage_pool.tile([P, NT], f32)
                    eng.scalar_tensor_tensor(
                        st[:],
                        probs[i][:, j * NT : (j + 1) * NT],
                        recips[:, i : i + 1],
                        bias_t[:, j * NT : (j + 1) * NT],
                        mybir.AluOpType.mult,
                        mybir.AluOpType.add,
                    )
                    nc.sync.dma_start(out[:, i, j * NT : (j + 1) * NT], st[:])

            def row_stats(i, nslots=NTILES):
                nc.vector.reduce_sum(
                    ssum[:, i : i + 1],
                    sums[:, i * NTILES : i * NTILES + nslots],
                    axis=mybir.AxisListType.X,
                )
                nc.vector.reciprocal(recips[:, i : i + 1], ssum[:, i : i + 1])

            ALLJ = range(NTILES)

            def slot(s):
                return sums[:, s : s + 1]

            # Phase 1: j-outer so each W slab is used for all ph1 m-tiles as
            # soon as it lands.
            for i in range(ph1):
                probs[i] = probs_pool.tile([P, N], bf16, tag="probs", name=f"probs{i}")
            for j in ALLJ:
                for i in range(ph1):
                    mm_tile(i, j * NT, (j + 1) * NT, slot(i * NTILES + j))


            # Phase 2: i-outer; normalize of tile i overlaps later matmuls.
            # Emission order is load-bearing two ways: norm(3)/norm(0) must
            # head the DVE queue at window i=4 so probs slots A/D recycle in
            # time, while the remaining norms are spread mid-j-loop across
            # windows i=4..7 so no window's DVE queue (epilogue + norms)
            # overflows its 27.6us matmul budget.
            norm_at = {(4, 3): 1, (5, 1): 2, (5, 5): 4, (6, 3): 5, (7, 3): 6}
            for i in range(ph1, MT):
                probs[i] = probs_pool.tile([P, N], bf16, tag="probs", name=f"probs{i}")
                if i == ph1:
                    load_x(i + 1)  # slots freed when phase 1 released x0..x2
                if i + 2 <= MT - 1:
                    load_x(i + 2)
                base = i * NTILES
                for j in ALLJ:
                    mm_tile(i, j * NT, (j + 1) * NT, slot(base + j))
                    if (i, j) in norm_at:
                        normalize(norm_at[(i, j)], ALLJ, nc.vector)
                row_stats(i)
                if i == ph1:
                    normalize(i, ALLJ, nc.vector)
                    for b in range(ph1):
                        row_stats(b)
                    normalize(0, ALLJ, nc.vector)
            normalize(MT - 1, ALLJ, nc.vector)
    nc.compile()
    return nc


def pack_inputs(x, weight, bias, MC=MC, NT=NT):
    """Host-side shard + pack into the DMA-friendly layouts the kernel expects."""
    M, K = x.shape
    N = weight.shape[0]
    KO = K // P
    MT = MC // P
    NTILES = N // NT
    ncores = M // MC
    fp8 = ml_dtypes.float8_e4m3fn
    # wt[j, p, ko, n] = 64*weight[j*NT+n, ko*P+p]
    wt = np.ascontiguousarray(
        (weight * W_SCALE).astype(fp8).reshape(NTILES, NT, KO, P).transpose(0, 3, 2, 1)
    )
    bias_b = np.ascontiguousarray(
        np.broadcast_to(bias.astype(ml_dtypes.bfloat16)[None, :], (P, N))
    )
    in_maps = []
    for c in range(ncores):
        xs = x[c * MC : (c + 1) * MC].astype(fp8)
        # xt[i, p, ko, m] = x_core[i*P+m, ko*P+p]
        xtc = np.ascontiguousarray(xs.reshape(MT, P, KO, P).transpose(0, 3, 2, 1))
        in_maps.append({"xt": xtc, "wt": wt, "bias": bias_b})
    return in_maps


def unpack_outputs(results, MC=MC, N=FULL_N):
    outs = []
    for res in results:
        o = np.asarray(res["out"])  # [P, MT, N]
        outs.append(o.transpose(1, 0, 2).reshape(MC, N))
    return np.concatenate(outs, axis=0)


_CACHE = {}


def _get_nc():
    if "nc" not in _CACHE:
        _CACHE["nc"] = build_nc()
    return _CACHE["nc"]


def _ensure_trace_env():
    """The agent image's antenv lacks axon_hooks, so NTFF tracing silently
    degrades. Register the ctypes-based hook ourselves, and neuter the S3
    artifact upload (no bucket access here)."""
    try:
        from antenv.axon_hooks import get_axon_ntff_profile_hook  # noqa: F401
    except ImportError:
        import types

        import antenv
        from trn_agent_boot.trn_boot import _ntff_profile_via_ctypes

        mod = types.ModuleType("antenv.axon_hooks")
        state = {"hook": _ntff_profile_via_ctypes("/opt/axon/libaxon_pjrt.so")}
        mod.set_axon_ntff_profile_hook = lambda h: state.__setitem__("hook", h)
        mod.get_axon_ntff_profile_hook = lambda: state["hook"]
        sys.modules["antenv.axon_hooks"] = mod
        antenv.axon_hooks = mod
    import concourse.bass_utils as bu

    bu.upload_artifacts = lambda tmpdir: f"local://{tmpdir}"


def kernel(x, weight, bias, trace=False, fp8=True):
    if trace:
        _ensure_trace_env()
    nc = _get_nc()
    in_maps = pack_inputs(
        np.asarray(x, dtype=np.float32),
        np.asarray(weight, dtype=np.float32),
        np.asarray(bias, dtype=np.float32),
    )
    res = run_bass_kernel_spmd(nc, in_maps, core_ids=list(range(NCORES)), trace=trace)
    out = unpack_outputs(res.results)
    if trace:
        return out, res
    return out
